# revision 15
# baseline (speedup 1.0000x reference)
"""GCN encoder (GCNConv + PReLU) as a Bass/Tile kernel on 8 Trainium2 NeuronCores.

Math (matches PyG GCNConv with self-loops + symmetric norm, then PReLU):
    deg[i]  = in-degree of i over dst (+1 self loop)
    dinv    = 1/sqrt(deg)
    agg[d]  = sum_{e:(s->d)} dinv[s]*dinv[d] * x[s] + dinv[d]^2 * x[d]
    out     = PReLU(agg @ W.T + bias)

Distribution: dst-node sharding, core k owns nodes [k*6250, (k+1)*6250).

Per-core pipeline (dst-blocks of 128 nodes):
  - each core's 6250 dst nodes are packed into 49 blocks of <=128 by a 2D
    bin-packing pass on (lo-half, hi-half) in-degree, so nearly every
    (block, src-half) cell fits exactly 4 chunks of 128 edges (~398 chunks
    vs 490 under the natural contiguous assignment). Chunk capacities are
    shared across cores (one SPMD program); a few data-driven spill blocks
    get 5 chunks. Block-permuted output rows are un-permuted on the host.
  - the symmetric norm factorizes: dinv[src] is premultiplied into the
    shipped x halves on the host; dinv[dst] is applied on-chip as the
    per-partition scale of the PSUM->SBUF copy of each block's accumulator.
    The self-loop term ships as dinv*x rows and rides the same copy scale.
  - non-self edges are grouped by (dst-block, src-half) on the host and
    packed into 128-edge chunks; src rows are fetched in bf16 with
    `dma_gather` (int16 indices => x is split into two 25000-row halves).
    Gathers rotate over 4 SWDGE queues, GBLK=3 blocks per gather pair.
    Pad slots gather row 0; their Msel column is zero so they contribute 0.
  - the scatter-add runs on the PE: per chunk, a one-hot selection matrix
    Msel[e, d] = (d == dst_local[e]) turns it into one matmul per chunk,
    A[d, c] += Msel[e, d]^T @ gx[e, c], accumulated in PSUM. Msel tiles are
    PRECOMPUTED on the host (exact one-hot bf16) and streamed per group on
    the sync HWDGE queue -- building them on DVE cost ~550ns/chunk of
    vector-engine time (~220us/core), which made DVE a co-bottleneck with
    the gather descriptor generation on GpSimd. Streaming costs ~13MB/core
    of extra DMA but leaves the Pool engine as the only pacer.
  - A is transposed with the PE (128x128 via identity); the PSUM->SBUF
    copies are split between DVE and ACT so neither sits on the critical
    per-block chain. The weight matmul H = A^T W^T (+ ones^T bias when the
    bias is nonzero -- added after the dinv[dst] scale, so correct) then a
    single ACT parametric-relu writes the bf16 output tile.
  - the gather-descriptor generation on the Pool engine (SWDGE ucode,
    ~2-4.5ns/row) is the end-to-end pacer; everything else overlaps it.

Dtype knobs (env):
  GCN_SC_DT  = f32 | f32r | bf16   scatter path (gather + Msel + edge matmul)
  GCN_FIN_DT = f32 | f32r          weight matmul path
  GCN_OUT_DT = f32 | bf16          DRAM output tile
Defaults (bf16 gather/Msel/out, f32r weight matmul, GBLK=3, streamed Msel)
measure ~207us on hardware vs ~435us for the session-start baseline;
rel err ~2.3e-3 (bf16 quantization of x), tolerance 2e-2.
"""

import os
import numpy as np
from contextlib import ExitStack

import concourse.bass as bass
import concourse.tile as tile
from concourse import bacc, mybir, bass_utils
from concourse.masks import make_identity

# Problem shape (fixed by the harness contract).
N_NODES = 50000
N_EDGES = 400000
IN_CH = 256
HID = 512
NCORES = 8
NPC = N_NODES // NCORES  # dst nodes owned per core
P = 128
BPC = (NPC + P - 1) // P  # dst blocks per core
NPC_PAD = BPC * P

F32 = mybir.dt.float32
BF16 = mybir.dt.bfloat16
# blocks whose gathers are merged into one dma_gather pair (lo/hi)
GBLK = int(os.environ.get("GCN_GBLK", "3"))
# of every 8 Msel builds, this many go to the scalar engine (rest on vector)
MSACT = int(os.environ.get("GCN_MSACT", "0"))
# pack each dma_gather's descriptors into one packet (crashes NRT for big
# gathers — exceeds the 64-descriptor packet ceiling; keep 0)
SINGLE_PACKET = os.environ.get("GCN_SINGLE_PACKET", "0") == "1"
# number of SWDGE queues to rotate gathers over (ucode max 4)
NQUEUES = int(os.environ.get("GCN_NQ", "4"))
# use the ACT engine's parametric relu (1 op) for PReLU
PRELU_ACT = os.environ.get("GCN_PRELU_ACT", "1") == "1"
# dtype of the DRAM output tile (bf16 halves store traffic; host re-widens)
OUT_DT = {"f32": F32, "bf16": BF16}[os.environ.get("GCN_OUT_DT", "bf16")]
# stream host-precomputed one-hot Msel tiles instead of building them on DVE
MSPRE = os.environ.get("GCN_MSPRE", "1") == "1"


def _group_plan(bpc):
    """Blocks per gather group: small groups at both ends (fast pipeline
    fill, little leftover work after the last gather), GBLK-sized in the
    middle. Shared by _preprocess and _build_program."""
    if os.environ.get("GCN_TAPER", "1") != "1" or GBLK <= 1:
        return [min(GBLK, bpc - i) for i in range(0, bpc, GBLK)]
    head = [1, 1, 2]
    tail = [2, 1, 1]
    mid_n = bpc - sum(head) - sum(tail)
    if mid_n <= 0:
        return [min(GBLK, bpc - i) for i in range(0, bpc, GBLK)]
    mid = [GBLK] * (mid_n // GBLK)
    if mid_n % GBLK:
        mid.append(mid_n % GBLK)
    return head + mid + tail


def _gx_bufs(sc_mm_dt):
    env = os.environ.get("GCN_GXBUFS")
    if env:
        return int(env)
    return 4 if sc_mm_dt == BF16 else max(2, 8 // GBLK)


# SWDGE descriptor-ring carveout in SBUF (bytes)
DMA_SCRATCH = int(os.environ.get("GCN_DMA_SCRATCH", "32768"))


def _pack_core(dlo, dhi, caplo, caphi, sizes):
    """Assign nodes (with per-half degree pairs) to blocks with per-block
    degree capacities and seat counts. Snake init + greedy swap repair.
    Returns blk[node] or None if the repair got stuck."""
    npc = len(dlo)
    nb = len(caplo)
    order = np.argsort(-(dlo + dhi), kind="stable")
    blk = np.empty(npc, np.int32)
    seat_left = sizes.copy()
    bi, direction = 0, 1
    for n in order:
        for _ in range(2 * nb):
            if seat_left[bi] > 0:
                break
            bi += direction
            if bi == nb:
                bi, direction = nb - 1, -1
            elif bi < 0:
                bi, direction = 0, 1
        blk[n] = bi
        seat_left[bi] -= 1
        bi += direction
        if bi == nb:
            bi, direction = nb - 1, -1
        elif bi < 0:
            bi, direction = 0, 1
    for _ in range(6000):
        slo = np.bincount(blk, weights=dlo, minlength=nb)
        shi = np.bincount(blk, weights=dhi, minlength=nb)
        vlo = slo - caplo
        vhi = shi - caphi
        viol = np.maximum(vlo, 0) + np.maximum(vhi, 0)
        if viol.max() <= 0:
            return blk
        B = int(np.argmax(viol))
        in_b = np.where(blk == B)[0]
        w_u = dlo[in_b] * (vlo[B] > 0) + dhi[in_b] * (vhi[B] > 0)
        u = in_b[np.argmax(w_u)]
        cand = np.where(blk != B)[0]
        C = blk[cand]
        dl_u, dh_u = dlo[u], dhi[u]
        nlo_b = slo[B] - dl_u + dlo[cand]
        nhi_b = shi[B] - dh_u + dhi[cand]
        nlo_c = slo[C] + dl_u - dlo[cand]
        nhi_c = shi[C] + dh_u - dhi[cand]
        newv = (
            np.maximum(nlo_b - caplo[B], 0)
            + np.maximum(nhi_b - caphi[B], 0)
            + np.maximum(nlo_c - caplo[C], 0)
            + np.maximum(nhi_c - caphi[C], 0)
        )
        gain = (viol[B] + viol[C]) - newv
        j = int(np.argmax(gain))
        if gain[j] <= 0:
            return None
        v = cand[j]
        blk[u], blk[v] = blk[v], B
    return None


def _preprocess(edge_index, n_nodes=N_NODES, ncores=NCORES):
    """Balance dst nodes into blocks, then group non-self edges by
    (core, dst-block, src-half) and pack into 128-edge chunks (capacities
    shared across cores so all cores run one program).

    Returns (klo, khi, idx16, dstl, nrm, dinv, gperm):
      klo/khi: per-block chunk counts for the lo/hi gathers (compile-time)
      idx16:   [ncores, 128, 8*tot] int16 gather indices (16-wrap, 8x tiled)
      dstl:    [ncores, 128, tot] f32 dst position-in-block per edge slot
      nrm:     [ncores, 128, tot] f32 edge norm (0 on padded slots)
      dinv:    [n_nodes] f32 1/sqrt(deg)
      gperm:   [n_nodes] int64; node v's output row is concat-out[gperm[v]]
    """
    npc = n_nodes // ncores
    half = n_nodes // 2
    bpc = (npc + P - 1) // P
    src = np.asarray(edge_index[0]).astype(np.int64).ravel()
    dst = np.asarray(edge_index[1]).astype(np.int64).ravel()
    deg = np.bincount(dst, minlength=n_nodes).astype(np.float32) + 1.0
    dinv = (1.0 / np.sqrt(deg)).astype(np.float32)
    n_all = dinv[src] * dinv[dst]

    core = dst // npc
    dloc = dst - core * npc
    hi = (src >= half).astype(np.int64)

    # per-core degree pairs over (lo, hi) src halves
    dlo_all = np.zeros((ncores, npc), np.int64)
    dhi_all = np.zeros((ncores, npc), np.int64)
    np.add.at(dlo_all, (core[hi == 0], dloc[hi == 0]), 1)
    np.add.at(dhi_all, (core[hi == 1], dloc[hi == 1]), 1)

    sizes = np.full(bpc, P, np.int64)
    extra = bpc * P - npc
    if extra:
        sizes[-extra:] = P - 1  # tail blocks hold one fewer node

    tmax = max(int(dlo_all.sum(1).max()), int(dhi_all.sum(1).max()))
    spill = max(0, -(-(tmax - bpc * 4 * P) // P)) + 1
    while True:
        caps = np.full(bpc, 4, np.int64)
        for i in range(spill):
            caps[i % bpc] += 1
        caplo = caps * P
        caphi = caps * P
        blks = []
        for k in range(ncores):
            b = _pack_core(dlo_all[k], dhi_all[k], caplo, caphi, sizes)
            if b is None:
                break
            blks.append(b)
        if len(blks) == ncores:
            break
        spill += 2
    klo = [int(c) for c in caps]
    khi = [int(c) for c in caps]

    # node -> (block, position) and the global output permutation
    blk_of = np.empty(n_nodes, np.int64)
    pos_of = np.empty(n_nodes, np.int64)
    for k in range(ncores):
        b = blks[k].astype(np.int64)
        order = np.lexsort((np.arange(npc), b))
        pos = np.empty(npc, np.int64)
        bsorted = b[order]
        start = np.searchsorted(bsorted, np.arange(bpc))
        pos[order] = np.arange(npc) - start[bsorted]
        blk_of[k * npc : (k + 1) * npc] = b
        pos_of[k * npc : (k + 1) * npc] = pos
    gperm = (dst_core := np.arange(n_nodes) // npc) * (bpc * P) + blk_of * P + pos_of
    del dst_core

    kblk = [a + b for a, b in zip(klo, khi)]
    tot = sum(kblk)

    blk = blk_of[dst]
    key = (core * bpc + blk) * 2 + hi
    nkeys = ncores * bpc * 2
    counts = np.bincount(key, minlength=nkeys).reshape(ncores, bpc, 2)
    assert (counts[:, :, 0] <= caplo[None, :]).all()
    assert (counts[:, :, 1] <= caphi[None, :]).all()

    order = np.argsort(key, kind="stable")
    key_sorted = key[order]
    grp_start = np.zeros(nkeys + 1, np.int64)
    grp_start[1:] = np.cumsum(counts.reshape(-1))
    rank = np.arange(len(key_sorted)) - grp_start[key_sorted]

    # chunk layout groups GBLK consecutive blocks per gather pair:
    # [lo(b0) lo(b1) .. | hi(b0) hi(b1) ..] per group, groups consecutive
    segbase = np.zeros((bpc, 2), np.int64)
    off = 0
    g0 = 0
    for gsz in _group_plan(bpc):
        blocks = range(g0, g0 + gsz)
        g0 += gsz
        for b in blocks:
            segbase[b, 0] = off
            off += klo[b]
        for b in blocks:
            segbase[b, 1] = off
            off += khi[b]
    assert off == tot

    ob, oh, oc = blk[order], hi[order], core[order]
    base = segbase[ob, oh]
    ck = base + rank // P
    pp = rank % P

    # pad slots keep dstl=200 (matches no iota value -> zero Msel column);
    # edge norms factorize: dinv[src] is folded into the shipped x, dinv[dst]
    # into the per-block PSUM->SBUF copy scale (dsc).
    dstl = np.full((ncores, P, tot), 200.0, np.float32)
    dstl[oc, pp, ck] = pos_of[dst[order]].astype(np.float32)
    dsc = np.zeros((ncores, P, bpc), np.float32)
    nodes = np.arange(n_nodes)
    dsc[nodes // npc, pos_of, blk_of] = dinv
    if MSPRE:
        # dense one-hot Msel, streamed per group at runtime
        import ml_dtypes
        msall = np.zeros((ncores, P, tot * P), ml_dtypes.bfloat16)
        msall[oc, pp, ck * P + pos_of[dst[order]]] = 1.0
    else:
        msall = None

    del n_all
    s16 = (src[order] - oh * half).astype(np.int16)
    col = 8 * base + (rank // 16)
    row = rank % 16
    idx16 = np.zeros((ncores, 16, 8 * tot), np.int16)  # pads gather row 0
    idx16[oc, row, col] = s16
    idx16 = np.tile(idx16, (1, 8, 1))
    return klo, khi, idx16, (dstl, msall), dsc, dinv, gperm


def _build_program(
    klo,
    khi,
    alpha,
    sc_dt=F32,
    sc_mm_dt=None,
    fin_mm_dt=None,
    n_nodes=N_NODES,
    ncores=NCORES,
    in_ch=IN_CH,
    hid=HID,
    has_bias=True,
):
    """Build the per-core Bass program (identical across cores).

    sc_dt: storage dtype of gather/Msel tiles (F32 or BF16).
    sc_mm_dt: dtype the scatter matmul sees (defaults to sc_dt; use
        mybir.dt.float32r with sc_dt=F32 for fast near-fp32 matmuls).
    fin_mm_dt: dtype of the weight matmul (F32 or float32r).
    """
    dblk = P
    npc = n_nodes // ncores
    half = n_nodes // 2
    bpc = len(klo)
    kblk = [a + b for a, b in zip(klo, khi)]
    tot = sum(kblk)
    nch = in_ch // P
    npc_pad = bpc * dblk
    sc_mm_dt = sc_mm_dt or sc_dt
    fin_mm_dt = fin_mm_dt or F32

    def fin_cast(ap):
        return ap

    nc = bacc.Bacc(
        "TRN2", target_bir_lowering=False, debug=False,
        num_swdge_queues=NQUEUES, dynamic_dma_scratch_size=DMA_SCRATCH,
    )

    x_ds = [
        nc.dram_tensor(f"x{h}", [half, in_ch], sc_mm_dt, kind="ExternalInput")
        for h in range(2)
    ]
    si_d = nc.dram_tensor("idx16", [P, 8 * tot], mybir.dt.int16, kind="ExternalInput")
    if MSPRE:
        ms_d = nc.dram_tensor("msall", [P, tot * P], sc_dt, kind="ExternalInput")
    else:
        dl_d = nc.dram_tensor("dstl", [P, tot], F32, kind="ExternalInput")
        dln_d = nc.dram_tensor("dlneg", [P, tot], F32, kind="ExternalInput")
    dsc_d = nc.dram_tensor("dsc", [P, bpc], F32, kind="ExternalInput")
    io_d = nc.dram_tensor("iota", [P, dblk], sc_mm_dt, kind="ExternalInput")
    xs_d = nc.dram_tensor("xself", [npc_pad, in_ch], sc_mm_dt, kind="ExternalInput")
    wt_ds = [
        nc.dram_tensor(f"wt{h}", [P, hid], fin_mm_dt, kind="ExternalInput")
        for h in range(nch)
    ]
    bs_d = nc.dram_tensor("bias", [1, hid], fin_mm_dt, kind="ExternalInput")
    on_d = nc.dram_tensor("ones", [1, P], fin_mm_dt, kind="ExternalInput")
    idr_d = nc.dram_tensor("idr", [P, P], sc_mm_dt, kind="ExternalInput")
    out_d = nc.dram_tensor("out", [npc_pad, hid], OUT_DT, kind="ExternalOutput")

    with tile.TileContext(nc) as tc, ExitStack() as ctx:
        const = ctx.enter_context(tc.tile_pool(name="const", bufs=1))
        gx_bufs = _gx_bufs(sc_mm_dt)
        gxp = ctx.enter_context(tc.tile_pool(name="gx", bufs=gx_bufs))
        mselp = ctx.enter_context(tc.tile_pool(name="msel", bufs=8))
        psA = ctx.enter_context(tc.tile_pool(name="psA", bufs=3, space="PSUM"))
        psT = ctx.enter_context(tc.tile_pool(name="psT", bufs=1, space="PSUM"))
        hps = ctx.enter_context(tc.tile_pool(name="hps", bufs=3, space="PSUM"))
        aS = ctx.enter_context(tc.tile_pool(name="aS", bufs=4))
        xsp = ctx.enter_context(tc.tile_pool(name="xsp", bufs=4))
        outp = ctx.enter_context(tc.tile_pool(name="outp", bufs=6))

        # column-chunked const loads: subtile deps let the first gathers and
        # Msel builds start as soon as their slice lands, not the whole 1MB
        def _chunked_load(t, dsrc, ncols, npiece=8, taper=False):
            bounds = [0]
            if taper:
                # first pieces tiny: the first gathers only need a sliver
                bounds += [max(1, ncols // 64), max(2, ncols // 16)]
            step = -(-(ncols - bounds[-1]) // npiece)
            a = bounds[-1]
            while a < ncols:
                a = min(a + step, ncols)
                bounds.append(a)
            for a, b in zip(bounds, bounds[1:]):
                if b > a:
                    nc.sync.dma_start(out=t[:, a:b], in_=dsrc.ap()[:, a:b])

        si_t = const.tile([P, 8 * tot], mybir.dt.int16)
        _chunked_load(si_t, si_d, 8 * tot, taper=True)
        if not MSPRE:
            dl_t = const.tile([P, tot], F32)
            _chunked_load(dl_t, dl_d, tot)
            if MSACT > 0:
                dln_t = const.tile([P, tot], F32)
                _chunked_load(dln_t, dln_d, tot)
        dsc_t = const.tile([P, bpc], F32)
        nc.sync.dma_start(out=dsc_t[:], in_=dsc_d.ap())
        io_t = const.tile([P, dblk], sc_mm_dt)
        nc.sync.dma_start(out=io_t[:], in_=io_d.ap())
        wt_t = []
        for h in range(nch):
            w = const.tile([P, hid], fin_mm_dt, name=f"wt_t{h}")
            nc.sync.dma_start(out=w[:], in_=wt_ds[h].ap())
            wt_t.append(w)
        bs_t = const.tile([1, hid], fin_mm_dt)
        nc.sync.dma_start(out=bs_t[:], in_=bs_d.ap())
        on_t = const.tile([1, P], fin_mm_dt)
        nc.sync.dma_start(out=on_t[:], in_=on_d.ap())
        id_t = const.tile([P, P], F32)
        make_identity(nc, id_t[:])
        idr_t = const.tile([P, P], sc_mm_dt)
        nc.sync.dma_start(out=idr_t[:], in_=idr_d.ap())

        # group-level chunk bases (same layout as _preprocess)
        segbase = np.zeros((bpc, 2), np.int64)
        off = 0
        groups = []
        g0 = 0
        for gsz in _group_plan(bpc):
            blocks = list(range(g0, g0 + gsz))
            g0 += gsz
            for b in blocks:
                segbase[b, 0] = off
                off += klo[b]
            for b in blocks:
                segbase[b, 1] = off
                off += khi[b]
            groups.append(blocks)

        # fixed slab extent so the pool rotates uniform buffers
        gmax = [
            max(sum((klo, khi)[h][b] for b in blocks) for blocks in groups)
            for h in range(2)
        ]

        gather_qn = 0
        for gri, blocks in enumerate(groups):
            b0 = blocks[0]
            kg = [sum(klo[b] for b in blocks), sum(khi[b] for b in blocks)]
            gstart = [int(segbase[b0, 0]), int(segbase[b0, 1])]
            gxs = []
            for h in range(2):
                if kg[h] == 0:
                    gxs.append(None)
                    continue
                nidx = kg[h] * P
                gx = gxp.tile(
                    [P, gmax[h] * in_ch], sc_mm_dt, tag=f"gx{h}", name=f"gx{h}_{b0}"
                )
                nc.gpsimd.dma_gather(
                    gx[:, : kg[h] * in_ch].rearrange("p (k d) -> p k d", d=in_ch),
                    x_ds[h].ap(),
                    si_t[:, 8 * gstart[h] : 8 * (gstart[h] + kg[h])],
                    nidx,
                    nidx,
                    in_ch,
                    queue_num=gather_qn % NQUEUES,
                    single_packet=SINGLE_PACKET,
                )
                gather_qn += 1
                gxs.append(gx)
            if MSPRE:
                # one streamed load covers the group's lo+hi chunk run
                g_c0 = gstart[0]
                g_nc = kg[0] + kg[1]
                msg = mselp.tile(
                    [P, g_nc * P], sc_dt, tag="msg", name=f"msg_{b0}"
                )
                nc.scalar.dma_start(
                    out=msg[:], in_=ms_d.ap()[:, g_c0 * P : (g_c0 + g_nc) * P]
                )
            for b in blocks:
                A = psA.tile([P, in_ch], F32, tag="A", name=f"A_{b}")
                first = True
                for h in range(2):
                    gx = gxs[h]
                    koff = int(segbase[b, h]) - gstart[h]
                    kk = (klo, khi)[h][b]
                    for j in range(kk):
                        ci = int(segbase[b, h]) + j
                        jj = koff + j
                        if MSPRE:
                            ms = msg[:, (ci - g_c0) * P : (ci - g_c0 + 1) * P]
                            nc.tensor.matmul(
                                A[:],
                                lhsT=ms,
                                rhs=gx[:, jj * in_ch : (jj + 1) * in_ch],
                                start=first,
                                stop=False,
                            )
                            first = False
                            continue
                        ms = mselp.tile(
                            [P, dblk], sc_mm_dt, tag="ms", name=f"ms_{b}_{h}_{j}"
                        )
                        if ci % 8 < MSACT:
                            # ScalarE build: ms = relu(1 - |iota - dst|)
                            mt = mselp.tile(
                                [P, dblk], sc_mm_dt, tag="mt", name=f"mt_{b}_{h}_{j}"
                            )
                            nc.scalar.activation(
                                out=mt[:],
                                in_=io_t[:],
                                func=mybir.ActivationFunctionType.Abs,
                                bias=dln_t[:, ci : ci + 1],
                            )
                            nc.scalar.activation(
                                out=ms[:],
                                in_=mt[:],
                                func=mybir.ActivationFunctionType.Relu,
                                scale=-1.0,
                                bias=1.0,
                            )
                        else:
                            nc.vector.tensor_scalar(
                                out=ms[:],
                                in0=io_t[:],
                                scalar1=dl_t[:, ci : ci + 1],
                                scalar2=None,
                                op0=mybir.AluOpType.is_equal,
                            )
                        nc.tensor.matmul(
                            A[:],
                            lhsT=ms[:],
                            rhs=gx[:, jj * in_ch : (jj + 1) * in_ch],
                            start=first,
                            stop=False,
                        )
                        first = False
                # A[d, c] += dinv[d]^2 * x[d, c] (host-prescaled), via identity mm
                xs_t = xsp.tile([P, in_ch], sc_mm_dt, tag="xs", name=f"xs_{b}")
                # ACT's HWDGE queue: splits hw-dynamic traffic off the sync queue
                nc.scalar.dma_start(
                    out=xs_t[:], in_=xs_d.ap()[b * dblk : (b + 1) * dblk, :]
                )
                nc.tensor.matmul(
                    A[:], lhsT=idr_t[:], rhs=xs_t[:], start=first, stop=True
                )
                a_s = aS.tile([P, in_ch], F32, tag="as", name=f"as_{b}")
                # PSUM->SBUF copy doubles as the dinv[dst] scale (norm factor);
                # runs on DVE (idle when Msel tiles are streamed) to keep the
                # ACT engine off the per-block critical chain
                if MSPRE:
                    nc.vector.tensor_scalar(
                        out=a_s[:],
                        in0=A[:],
                        scalar1=dsc_t[:, b : b + 1],
                        scalar2=None,
                        op0=mybir.AluOpType.mult,
                    )
                else:
                    nc.scalar.activation(
                        out=a_s[:],
                        in_=A[:],
                        func=mybir.ActivationFunctionType.Copy,
                        scale=dsc_t[:, b : b + 1],
                    )
                # transpose A halves on the PE: AT[c, d] = A[d, c]^T
                at_s = []
                for h in range(nch):
                    atp = psT.tile([P, P], F32, tag=f"atp{h}", name=f"atp{h}_{b}")
                    nc.tensor.transpose(
                        out=atp[:], in_=a_s[:, h * P : (h + 1) * P], identity=id_t[:]
                    )
                    ats = aS.tile([P, P], fin_mm_dt, tag=f"ats{h}", name=f"ats{h}_{b}")
                    if MSPRE and h == 0:
                        nc.vector.tensor_scalar(
                            out=ats[:], in0=atp[:], scalar1=1.0, scalar2=None,
                            op0=mybir.AluOpType.mult,
                        )
                    else:
                        nc.scalar.copy(ats[:], atp[:])
                    at_s.append(ats)
                Hp = hps.tile([P, hid], F32, tag="hp", name=f"hp_{b}")
                for h in range(nch):
                    nc.tensor.matmul(
                        Hp[:],
                        lhsT=fin_cast(at_s[h][:]),
                        rhs=fin_cast(wt_t[h][:]),
                        start=(h == 0),
                        stop=(not has_bias and h == nch - 1),
                    )
                if has_bias:
                    nc.tensor.matmul(
                        Hp[:],
                        lhsT=fin_cast(on_t[:]),
                        rhs=fin_cast(bs_t[:]),
                        start=False,
                        stop=True,
                    )
                os_ = outp.tile([P, hid], OUT_DT, tag="os", name=f"os_{b}")
                t2 = (
                    None
                    if PRELU_ACT
                    else outp.tile([P, hid], F32, tag="t2", name=f"t2_{b}")
                )
                if PRELU_ACT:
                    # single ACT-engine parametric relu: x>0 ? x : alpha*x
                    nc.scalar.activation(
                        out=os_[:],
                        in_=Hp[:],
                        func=mybir.ActivationFunctionType.Prelu,
                        alpha=float(alpha),
                    )
                elif 0.0 <= alpha <= 1.0:
                    # PReLU = max(H, alpha*H)
                    nc.scalar.activation(
                        out=t2[:],
                        in_=Hp[:],
                        func=mybir.ActivationFunctionType.Copy,
                        scale=float(alpha),
                    )
                    nc.vector.tensor_tensor(
                        out=os_[:], in0=t2[:], in1=Hp[:], op=mybir.AluOpType.max
                    )
                else:
                    # general PReLU: relu(H)*(1-alpha) + alpha*H
                    nc.scalar.activation(
                        out=t2[:],
                        in_=Hp[:],
                        func=mybir.ActivationFunctionType.Relu,
                    )
                    nc.vector.tensor_scalar(
                        out=t2[:],
                        in0=t2[:],
                        scalar1=float(1.0 - alpha),
                        scalar2=None,
                        op0=mybir.AluOpType.mult,
                    )
                    t3 = outp.tile([P, hid], F32, tag="t3", name=f"t3_{b}")
                    nc.vector.tensor_scalar(
                        out=t3[:],
                        in0=Hp[:],
                        scalar1=float(alpha),
                        scalar2=None,
                        op0=mybir.AluOpType.mult,
                    )
                    nc.vector.tensor_tensor(
                        out=os_[:], in0=t2[:], in1=t3[:], op=mybir.AluOpType.add
                    )
                row0 = b * dblk
                nc.sync.dma_start(
                    out=out_d.ap()[row0 : row0 + dblk, :], in_=os_[:, :]
                )
    nc.compile()
    return nc


def _make_in_maps(
    x, weight, bias, idx16, dstl, dsc, dinv, gperm,
    sc_np=np.float32, ncores=NCORES,
):
    dstl, msall = dstl
    x = np.asarray(x, dtype=np.float32)
    w = np.asarray(weight, dtype=np.float32)
    n = x.shape[0]
    half = n // 2
    in_ch = x.shape[1]
    hid = w.shape[0]
    npc = n // ncores
    bpc = (npc + P - 1) // P
    npc_pad = bpc * P
    iota = np.tile(np.arange(P, dtype=sc_np), (P, 1))
    wts = {
        f"wt{h}": np.ascontiguousarray(w[:, h * P : (h + 1) * P].T)
        for h in range(in_ch // P)
    }
    bias_row = np.asarray(bias, dtype=np.float32).reshape(1, hid)
    xp = x * dinv[:, None]  # dinv[src] premultiplied into the gathered rows
    xlo = np.ascontiguousarray(xp[:half].astype(sc_np))
    xhi = np.ascontiguousarray(xp[half:].astype(sc_np))
    xself_all = xp  # self term: dinv*x here, the copy scale adds dinv[dst]
    in_maps = []
    for k in range(ncores):
        xs = np.zeros((npc_pad, in_ch), sc_np)
        loc = gperm[k * npc : (k + 1) * npc] - k * npc_pad
        xs[loc] = xself_all[k * npc : (k + 1) * npc].astype(sc_np)
        m = {
            "x0": xlo,
            "x1": xhi,
            "idx16": np.ascontiguousarray(idx16[k]),
            "dsc": np.ascontiguousarray(dsc[k]),
            "iota": iota,
            "xself": xs,
            "bias": bias_row,
            "ones": np.ones((1, P), np.float32),
            "idr": np.eye(P, dtype=sc_np),
        }
        if msall is not None:
            m["msall"] = np.ascontiguousarray(msall[k])
        else:
            m["dstl"] = np.ascontiguousarray(dstl[k])
            m["dlneg"] = np.ascontiguousarray(-dstl[k])
        m.update(wts)
        in_maps.append(m)
    return in_maps


# Results of the last kernel() call, for the test harness.
LAST_RESULTS = None


def _dt_opts():
    sc = os.environ.get("GCN_SC_DT", "bf16")
    fin = os.environ.get("GCN_FIN_DT", "f32r")
    sc_dt = {"f32": F32, "f32r": F32, "bf16": BF16}[sc]
    sc_mm_dt = {"f32": F32, "f32r": mybir.dt.float32r, "bf16": BF16}[sc]
    fin_mm_dt = {"f32": F32, "f32r": mybir.dt.float32r}[fin]
    sc_np = np.float32 if sc_dt == F32 else mybir.dt.np(BF16)
    return sc_dt, sc_mm_dt, fin_mm_dt, sc_np


def kernel(x, edge_index, weight, bias, prelu_a):
    global LAST_RESULTS
    sc_dt, sc_mm_dt, fin_mm_dt, sc_np = _dt_opts()
    trace = os.environ.get("GCN_TRACE", "0") == "1"

    klo, khi, idx16, dstl, dsc, dinv, gperm = _preprocess(edge_index)
    alpha = float(np.asarray(prelu_a).ravel()[0])
    has_bias = bool(np.any(np.asarray(bias)))
    nc = _build_program(
        klo, khi, alpha, sc_dt=sc_dt, sc_mm_dt=sc_mm_dt, fin_mm_dt=fin_mm_dt,
        has_bias=has_bias,
    )
    in_maps = _make_in_maps(
        x, weight, bias, idx16, dstl, dsc, dinv, gperm, sc_np=sc_np
    )

    res = bass_utils.run_bass_kernel_spmd(
        nc, in_maps, core_ids=list(range(NCORES)), trace=trace
    )
    LAST_RESULTS = res
    out = np.concatenate([res.results[k]["out"] for k in range(NCORES)], axis=0)
    return out[gperm].astype(np.float32)


# revision 16
# speedup vs baseline: 1.2771x; 1.2771x over previous
"""GCN encoder (GCNConv + PReLU) as a Bass/Tile kernel on 8 Trainium2 NeuronCores.

Math (matches PyG GCNConv with self-loops + symmetric norm, then PReLU):
    deg[i]  = in-degree of i over dst (+1 self loop)
    dinv    = 1/sqrt(deg)
    agg[d]  = sum_{e:(s->d)} dinv[s]*dinv[d] * x[s] + dinv[d]^2 * x[d]
    out     = PReLU(agg @ W.T + bias)

Distribution: dst-node sharding, core k owns nodes [k*6250, (k+1)*6250).

Per-core pipeline (dst-blocks of 128 nodes):
  - each core's 6250 dst nodes are packed into 49 blocks of <=128 by a 2D
    bin-packing pass on (lo-half, hi-half) in-degree, so nearly every
    (block, src-half) cell fits exactly 4 chunks of 128 edges (~398 chunks
    vs 490 under the natural contiguous assignment). Chunk capacities are
    shared across cores (one SPMD program); a few data-driven spill blocks
    get 5 chunks. Block-permuted output rows are un-permuted on the host.
  - the symmetric norm factorizes: dinv[src] is premultiplied into the
    shipped x halves on the host; dinv[dst] is applied on-chip as the
    per-partition scale of the PSUM->SBUF copy of each block's accumulator.
    The self-loop term ships as dinv*x rows and rides the same copy scale.
  - non-self edges are grouped by (dst-block, src-half) on the host and
    packed into 128-edge chunks; src rows are fetched in bf16 with
    `dma_gather` (int16 indices => x is split into two 25000-row halves).
    Gathers rotate over 4 SWDGE queues, GBLK=3 blocks per gather pair.
    Pad slots gather row 0; their Msel column is zero so they contribute 0.
  - the scatter-add runs on the PE: per chunk, a one-hot selection matrix
    Msel[e, d] = (d == dst_local[e]) turns it into one matmul per chunk,
    A[d, c] += Msel[e, d]^T @ gx[e, c], accumulated in PSUM. Msel tiles are
    PRECOMPUTED on the host (exact one-hot bf16) and streamed per group on
    the sync HWDGE queue -- building them on DVE cost ~550ns/chunk of
    vector-engine time (~220us/core), which made DVE a co-bottleneck with
    the gather descriptor generation on GpSimd. Streaming costs ~13MB/core
    of extra DMA but leaves the Pool engine as the only pacer.
  - A is transposed with the PE (128x128 via identity); the PSUM->SBUF
    copies are split between DVE and ACT so neither sits on the critical
    per-block chain. The weight matmul H = A^T W^T (+ ones^T bias when the
    bias is nonzero -- added after the dinv[dst] scale, so correct) then a
    single ACT parametric-relu writes the bf16 output tile.
  - the gather-descriptor generation on the Pool engine (SWDGE ucode,
    ~2-4.5ns/row) is the end-to-end pacer; everything else overlaps it.

Dtype knobs (env):
  GCN_SC_DT  = f32 | f32r | bf16   scatter path (gather + Msel + edge matmul)
  GCN_FIN_DT = f32 | f32r          weight matmul path
  GCN_OUT_DT = f32 | bf16          DRAM output tile
Defaults (bf16 gather/Msel/out, f32r weight matmul, GBLK=3, streamed Msel)
measure ~207us on hardware vs ~435us for the session-start baseline;
rel err ~2.3e-3 (bf16 quantization of x), tolerance 2e-2.
"""

import os
import numpy as np
from contextlib import ExitStack

import concourse.bass as bass
import concourse.tile as tile
from concourse import bacc, mybir, bass_utils
from concourse.masks import make_identity

# Problem shape (fixed by the harness contract).
N_NODES = 50000
N_EDGES = 400000
IN_CH = 256
HID = 512
NCORES = 8
NPC = N_NODES // NCORES  # dst nodes owned per core
P = 128
BPC = (NPC + P - 1) // P  # dst blocks per core
NPC_PAD = BPC * P

F32 = mybir.dt.float32
BF16 = mybir.dt.bfloat16
# blocks whose gathers are merged into one dma_gather pair (lo/hi)
GBLK = int(os.environ.get("GCN_GBLK", "3"))
# of every 8 Msel builds, this many go to the scalar engine (rest on vector)
MSACT = int(os.environ.get("GCN_MSACT", "0"))
# pack each dma_gather's descriptors into one packet (crashes NRT for big
# gathers — exceeds the 64-descriptor packet ceiling; keep 0)
SINGLE_PACKET = os.environ.get("GCN_SINGLE_PACKET", "0") == "1"
# number of SWDGE queues to rotate gathers over (ucode max 4)
NQUEUES = int(os.environ.get("GCN_NQ", "4"))
# use the ACT engine's parametric relu (1 op) for PReLU
PRELU_ACT = os.environ.get("GCN_PRELU_ACT", "1") == "1"
# dtype of the DRAM output tile (bf16 halves store traffic; host re-widens)
OUT_DT = {"f32": F32, "bf16": BF16}[os.environ.get("GCN_OUT_DT", "bf16")]
# stream host-precomputed one-hot Msel tiles instead of building them on DVE
MSPRE = os.environ.get("GCN_MSPRE", "1") == "1"


def _group_plan(bpc):
    """Blocks per gather group: small groups at both ends (fast pipeline
    fill, little leftover work after the last gather), GBLK-sized in the
    middle. Shared by _preprocess and _build_program."""
    if os.environ.get("GCN_TAPER", "0") != "1" or GBLK <= 1:
        return [min(GBLK, bpc - i) for i in range(0, bpc, GBLK)]
    head = [1, 1, 2]
    tail = [2, 1, 1]
    mid_n = bpc - sum(head) - sum(tail)
    if mid_n <= 0:
        return [min(GBLK, bpc - i) for i in range(0, bpc, GBLK)]
    mid = [GBLK] * (mid_n // GBLK)
    if mid_n % GBLK:
        mid.append(mid_n % GBLK)
    return head + mid + tail


def _gx_bufs(sc_mm_dt):
    env = os.environ.get("GCN_GXBUFS")
    if env:
        return int(env)
    return 4 if sc_mm_dt == BF16 else max(2, 8 // GBLK)


# SWDGE descriptor-ring carveout in SBUF (bytes)
DMA_SCRATCH = int(os.environ.get("GCN_DMA_SCRATCH", "32768"))


def _pack_core(dlo, dhi, caplo, caphi, sizes):
    """Assign nodes (with per-half degree pairs) to blocks with per-block
    degree capacities and seat counts. Snake init + greedy swap repair.
    Returns blk[node] or None if the repair got stuck."""
    npc = len(dlo)
    nb = len(caplo)
    order = np.argsort(-(dlo + dhi), kind="stable")
    blk = np.empty(npc, np.int32)
    seat_left = sizes.copy()
    bi, direction = 0, 1
    for n in order:
        for _ in range(2 * nb):
            if seat_left[bi] > 0:
                break
            bi += direction
            if bi == nb:
                bi, direction = nb - 1, -1
            elif bi < 0:
                bi, direction = 0, 1
        blk[n] = bi
        seat_left[bi] -= 1
        bi += direction
        if bi == nb:
            bi, direction = nb - 1, -1
        elif bi < 0:
            bi, direction = 0, 1
    for _ in range(6000):
        slo = np.bincount(blk, weights=dlo, minlength=nb)
        shi = np.bincount(blk, weights=dhi, minlength=nb)
        vlo = slo - caplo
        vhi = shi - caphi
        viol = np.maximum(vlo, 0) + np.maximum(vhi, 0)
        if viol.max() <= 0:
            return blk
        B = int(np.argmax(viol))
        in_b = np.where(blk == B)[0]
        w_u = dlo[in_b] * (vlo[B] > 0) + dhi[in_b] * (vhi[B] > 0)
        u = in_b[np.argmax(w_u)]
        cand = np.where(blk != B)[0]
        C = blk[cand]
        dl_u, dh_u = dlo[u], dhi[u]
        nlo_b = slo[B] - dl_u + dlo[cand]
        nhi_b = shi[B] - dh_u + dhi[cand]
        nlo_c = slo[C] + dl_u - dlo[cand]
        nhi_c = shi[C] + dh_u - dhi[cand]
        newv = (
            np.maximum(nlo_b - caplo[B], 0)
            + np.maximum(nhi_b - caphi[B], 0)
            + np.maximum(nlo_c - caplo[C], 0)
            + np.maximum(nhi_c - caphi[C], 0)
        )
        gain = (viol[B] + viol[C]) - newv
        j = int(np.argmax(gain))
        if gain[j] <= 0:
            return None
        v = cand[j]
        blk[u], blk[v] = blk[v], B
    return None


def _preprocess(edge_index, n_nodes=N_NODES, ncores=NCORES):
    """Balance dst nodes into blocks, then group non-self edges by
    (core, dst-block, src-half) and pack into 128-edge chunks (capacities
    shared across cores so all cores run one program).

    Returns (klo, khi, idx16, dstl, nrm, dinv, gperm):
      klo/khi: per-block chunk counts for the lo/hi gathers (compile-time)
      idx16:   [ncores, 128, 8*tot] int16 gather indices (16-wrap, 8x tiled)
      dstl:    [ncores, 128, tot] f32 dst position-in-block per edge slot
      nrm:     [ncores, 128, tot] f32 edge norm (0 on padded slots)
      dinv:    [n_nodes] f32 1/sqrt(deg)
      gperm:   [n_nodes] int64; node v's output row is concat-out[gperm[v]]
    """
    npc = n_nodes // ncores
    half = n_nodes // 2
    bpc = (npc + P - 1) // P
    src = np.asarray(edge_index[0]).astype(np.int64).ravel()
    dst = np.asarray(edge_index[1]).astype(np.int64).ravel()
    deg = np.bincount(dst, minlength=n_nodes).astype(np.float32) + 1.0
    dinv = (1.0 / np.sqrt(deg)).astype(np.float32)
    n_all = dinv[src] * dinv[dst]

    core = dst // npc
    dloc = dst - core * npc
    hi = (src >= half).astype(np.int64)

    # per-core degree pairs over (lo, hi) src halves
    dlo_all = np.zeros((ncores, npc), np.int64)
    dhi_all = np.zeros((ncores, npc), np.int64)
    np.add.at(dlo_all, (core[hi == 0], dloc[hi == 0]), 1)
    np.add.at(dhi_all, (core[hi == 1], dloc[hi == 1]), 1)

    sizes = np.full(bpc, P, np.int64)
    extra = bpc * P - npc
    if extra:
        sizes[-extra:] = P - 1  # tail blocks hold one fewer node

    tmax = max(int(dlo_all.sum(1).max()), int(dhi_all.sum(1).max()))
    spill = max(0, -(-(tmax - bpc * 4 * P) // P)) + 1
    while True:
        caps = np.full(bpc, 4, np.int64)
        for i in range(spill):
            caps[i % bpc] += 1
        caplo = caps * P
        caphi = caps * P
        blks = []
        for k in range(ncores):
            b = _pack_core(dlo_all[k], dhi_all[k], caplo, caphi, sizes)
            if b is None:
                break
            blks.append(b)
        if len(blks) == ncores:
            break
        spill += 2
    klo = [int(c) for c in caps]
    khi = [int(c) for c in caps]

    # node -> (block, position) and the global output permutation
    blk_of = np.empty(n_nodes, np.int64)
    pos_of = np.empty(n_nodes, np.int64)
    for k in range(ncores):
        b = blks[k].astype(np.int64)
        order = np.lexsort((np.arange(npc), b))
        pos = np.empty(npc, np.int64)
        bsorted = b[order]
        start = np.searchsorted(bsorted, np.arange(bpc))
        pos[order] = np.arange(npc) - start[bsorted]
        blk_of[k * npc : (k + 1) * npc] = b
        pos_of[k * npc : (k + 1) * npc] = pos
    gperm = (dst_core := np.arange(n_nodes) // npc) * (bpc * P) + blk_of * P + pos_of
    del dst_core

    kblk = [a + b for a, b in zip(klo, khi)]
    tot = sum(kblk)

    blk = blk_of[dst]
    key = (core * bpc + blk) * 2 + hi
    nkeys = ncores * bpc * 2
    counts = np.bincount(key, minlength=nkeys).reshape(ncores, bpc, 2)
    assert (counts[:, :, 0] <= caplo[None, :]).all()
    assert (counts[:, :, 1] <= caphi[None, :]).all()

    order = np.argsort(key, kind="stable")
    key_sorted = key[order]
    grp_start = np.zeros(nkeys + 1, np.int64)
    grp_start[1:] = np.cumsum(counts.reshape(-1))
    rank = np.arange(len(key_sorted)) - grp_start[key_sorted]

    # chunk layout groups GBLK consecutive blocks per gather pair:
    # [lo(b0) lo(b1) .. | hi(b0) hi(b1) ..] per group, groups consecutive
    segbase = np.zeros((bpc, 2), np.int64)
    off = 0
    g0 = 0
    for gsz in _group_plan(bpc):
        blocks = range(g0, g0 + gsz)
        g0 += gsz
        for b in blocks:
            segbase[b, 0] = off
            off += klo[b]
        for b in blocks:
            segbase[b, 1] = off
            off += khi[b]
    assert off == tot

    ob, oh, oc = blk[order], hi[order], core[order]
    base = segbase[ob, oh]
    ck = base + rank // P
    pp = rank % P

    # pad slots keep dstl=200 (matches no iota value -> zero Msel column);
    # edge norms factorize: dinv[src] is folded into the shipped x, dinv[dst]
    # into the per-block PSUM->SBUF copy scale (dsc).
    dstl = np.full((ncores, P, tot), 200.0, np.float32)
    dstl[oc, pp, ck] = pos_of[dst[order]].astype(np.float32)
    dsc = np.zeros((ncores, P, bpc), np.float32)
    nodes = np.arange(n_nodes)
    dsc[nodes // npc, pos_of, blk_of] = dinv
    if MSPRE:
        # dense one-hot Msel, streamed per group at runtime
        import ml_dtypes
        msall = np.zeros((ncores, P, tot * P), ml_dtypes.bfloat16)
        msall[oc, pp, ck * P + pos_of[dst[order]]] = 1.0
    else:
        msall = None

    del n_all
    s16 = (src[order] - oh * half).astype(np.int16)
    col = 8 * base + (rank // 16)
    row = rank % 16
    idx16 = np.zeros((ncores, 16, 8 * tot), np.int16)  # pads gather row 0
    idx16[oc, row, col] = s16
    idx16 = np.tile(idx16, (1, 8, 1))
    return klo, khi, idx16, (dstl, msall), dsc, dinv, gperm


def _build_program(
    klo,
    khi,
    alpha,
    sc_dt=F32,
    sc_mm_dt=None,
    fin_mm_dt=None,
    n_nodes=N_NODES,
    ncores=NCORES,
    in_ch=IN_CH,
    hid=HID,
    has_bias=True,
):
    """Build the per-core Bass program (identical across cores).

    sc_dt: storage dtype of gather/Msel tiles (F32 or BF16).
    sc_mm_dt: dtype the scatter matmul sees (defaults to sc_dt; use
        mybir.dt.float32r with sc_dt=F32 for fast near-fp32 matmuls).
    fin_mm_dt: dtype of the weight matmul (F32 or float32r).
    """
    dblk = P
    npc = n_nodes // ncores
    half = n_nodes // 2
    bpc = len(klo)
    kblk = [a + b for a, b in zip(klo, khi)]
    tot = sum(kblk)
    nch = in_ch // P
    npc_pad = bpc * dblk
    sc_mm_dt = sc_mm_dt or sc_dt
    fin_mm_dt = fin_mm_dt or F32

    def fin_cast(ap):
        return ap

    nc = bacc.Bacc(
        "TRN2", target_bir_lowering=False, debug=False,
        num_swdge_queues=NQUEUES, dynamic_dma_scratch_size=DMA_SCRATCH,
    )

    x_ds = [
        nc.dram_tensor(f"x{h}", [half, in_ch], sc_mm_dt, kind="ExternalInput")
        for h in range(2)
    ]
    si_d = nc.dram_tensor("idx16", [P, 8 * tot], mybir.dt.int16, kind="ExternalInput")
    if MSPRE:
        ms_d = nc.dram_tensor("msall", [P, tot * P], sc_dt, kind="ExternalInput")
    else:
        dl_d = nc.dram_tensor("dstl", [P, tot], F32, kind="ExternalInput")
        dln_d = nc.dram_tensor("dlneg", [P, tot], F32, kind="ExternalInput")
    dsc_d = nc.dram_tensor("dsc", [P, bpc], F32, kind="ExternalInput")
    io_d = nc.dram_tensor("iota", [P, dblk], sc_mm_dt, kind="ExternalInput")
    xs_d = nc.dram_tensor("xself", [npc_pad, in_ch], sc_mm_dt, kind="ExternalInput")
    wt_ds = [
        nc.dram_tensor(f"wt{h}", [P, hid], fin_mm_dt, kind="ExternalInput")
        for h in range(nch)
    ]
    bs_d = nc.dram_tensor("bias", [1, hid], fin_mm_dt, kind="ExternalInput")
    on_d = nc.dram_tensor("ones", [1, P], fin_mm_dt, kind="ExternalInput")
    idr_d = nc.dram_tensor("idr", [P, P], sc_mm_dt, kind="ExternalInput")
    out_d = nc.dram_tensor("out", [npc_pad, hid], OUT_DT, kind="ExternalOutput")

    with tile.TileContext(nc) as tc, ExitStack() as ctx:
        const = ctx.enter_context(tc.tile_pool(name="const", bufs=1))
        gx_bufs = _gx_bufs(sc_mm_dt)
        gxp = ctx.enter_context(tc.tile_pool(name="gx", bufs=gx_bufs))
        mselp = ctx.enter_context(tc.tile_pool(name="msel", bufs=8))
        psA = ctx.enter_context(tc.tile_pool(name="psA", bufs=3, space="PSUM"))
        psT = ctx.enter_context(tc.tile_pool(name="psT", bufs=1, space="PSUM"))
        hps = ctx.enter_context(tc.tile_pool(name="hps", bufs=3, space="PSUM"))
        aS = ctx.enter_context(tc.tile_pool(name="aS", bufs=4))
        xsp = ctx.enter_context(tc.tile_pool(name="xsp", bufs=4))
        outp = ctx.enter_context(tc.tile_pool(name="outp", bufs=6))

        # column-chunked const loads: subtile deps let the first gathers and
        # Msel builds start as soon as their slice lands, not the whole 1MB
        def _chunked_load(t, dsrc, ncols, npiece=8, taper=False):
            bounds = [0]
            if taper:
                # first pieces tiny: the first gathers only need a sliver
                bounds += [max(1, ncols // 64), max(2, ncols // 16)]
            step = -(-(ncols - bounds[-1]) // npiece)
            a = bounds[-1]
            while a < ncols:
                a = min(a + step, ncols)
                bounds.append(a)
            for a, b in zip(bounds, bounds[1:]):
                if b > a:
                    nc.sync.dma_start(out=t[:, a:b], in_=dsrc.ap()[:, a:b])

        si_t = const.tile([P, 8 * tot], mybir.dt.int16)
        _chunked_load(si_t, si_d, 8 * tot,
                      taper=os.environ.get("GCN_SITAPER", "0") == "1")
        if not MSPRE:
            dl_t = const.tile([P, tot], F32)
            _chunked_load(dl_t, dl_d, tot)
            if MSACT > 0:
                dln_t = const.tile([P, tot], F32)
                _chunked_load(dln_t, dln_d, tot)
        dsc_t = const.tile([P, bpc], F32)
        nc.sync.dma_start(out=dsc_t[:], in_=dsc_d.ap())
        io_t = const.tile([P, dblk], sc_mm_dt)
        nc.sync.dma_start(out=io_t[:], in_=io_d.ap())
        wt_t = []
        for h in range(nch):
            w = const.tile([P, hid], fin_mm_dt, name=f"wt_t{h}")
            nc.sync.dma_start(out=w[:], in_=wt_ds[h].ap())
            wt_t.append(w)
        bs_t = const.tile([1, hid], fin_mm_dt)
        nc.sync.dma_start(out=bs_t[:], in_=bs_d.ap())
        on_t = const.tile([1, P], fin_mm_dt)
        nc.sync.dma_start(out=on_t[:], in_=on_d.ap())
        id_t = const.tile([P, P], F32)
        make_identity(nc, id_t[:])
        idr_t = const.tile([P, P], sc_mm_dt)
        nc.sync.dma_start(out=idr_t[:], in_=idr_d.ap())

        # group-level chunk bases (same layout as _preprocess)
        segbase = np.zeros((bpc, 2), np.int64)
        off = 0
        groups = []
        g0 = 0
        for gsz in _group_plan(bpc):
            blocks = list(range(g0, g0 + gsz))
            g0 += gsz
            for b in blocks:
                segbase[b, 0] = off
                off += klo[b]
            for b in blocks:
                segbase[b, 1] = off
                off += khi[b]
            groups.append(blocks)

        # fixed slab extent so the pool rotates uniform buffers
        gmax = [
            max(sum((klo, khi)[h][b] for b in blocks) for blocks in groups)
            for h in range(2)
        ]

        gather_qn = 0
        for gri, blocks in enumerate(groups):
            b0 = blocks[0]
            kg = [sum(klo[b] for b in blocks), sum(khi[b] for b in blocks)]
            gstart = [int(segbase[b0, 0]), int(segbase[b0, 1])]
            gxs = []
            for h in range(2):
                if kg[h] == 0:
                    gxs.append(None)
                    continue
                nidx = kg[h] * P
                gx = gxp.tile(
                    [P, gmax[h] * in_ch], sc_mm_dt, tag=f"gx{h}", name=f"gx{h}_{b0}"
                )
                nc.gpsimd.dma_gather(
                    gx[:, : kg[h] * in_ch].rearrange("p (k d) -> p k d", d=in_ch),
                    x_ds[h].ap(),
                    si_t[:, 8 * gstart[h] : 8 * (gstart[h] + kg[h])],
                    nidx,
                    nidx,
                    in_ch,
                    queue_num=gather_qn % NQUEUES,
                    single_packet=SINGLE_PACKET,
                )
                gather_qn += 1
                gxs.append(gx)
            if MSPRE:
                # one streamed load covers the group's lo+hi chunk run
                g_c0 = gstart[0]
                g_nc = kg[0] + kg[1]
                msg = mselp.tile(
                    [P, g_nc * P], sc_dt, tag="msg", name=f"msg_{b0}"
                )
                msq = nc.scalar if os.environ.get("GCN_MSGQ") == "act" else nc.sync
                msq.dma_start(
                    out=msg[:], in_=ms_d.ap()[:, g_c0 * P : (g_c0 + g_nc) * P]
                )
            for b in blocks:
                A = psA.tile([P, in_ch], F32, tag="A", name=f"A_{b}")
                first = True
                for h in range(2):
                    gx = gxs[h]
                    koff = int(segbase[b, h]) - gstart[h]
                    kk = (klo, khi)[h][b]
                    for j in range(kk):
                        ci = int(segbase[b, h]) + j
                        jj = koff + j
                        if MSPRE:
                            ms = msg[:, (ci - g_c0) * P : (ci - g_c0 + 1) * P]
                            nc.tensor.matmul(
                                A[:],
                                lhsT=ms,
                                rhs=gx[:, jj * in_ch : (jj + 1) * in_ch],
                                start=first,
                                stop=False,
                            )
                            first = False
                            continue
                        ms = mselp.tile(
                            [P, dblk], sc_mm_dt, tag="ms", name=f"ms_{b}_{h}_{j}"
                        )
                        if ci % 8 < MSACT:
                            # ScalarE build: ms = relu(1 - |iota - dst|)
                            mt = mselp.tile(
                                [P, dblk], sc_mm_dt, tag="mt", name=f"mt_{b}_{h}_{j}"
                            )
                            nc.scalar.activation(
                                out=mt[:],
                                in_=io_t[:],
                                func=mybir.ActivationFunctionType.Abs,
                                bias=dln_t[:, ci : ci + 1],
                            )
                            nc.scalar.activation(
                                out=ms[:],
                                in_=mt[:],
                                func=mybir.ActivationFunctionType.Relu,
                                scale=-1.0,
                                bias=1.0,
                            )
                        else:
                            nc.vector.tensor_scalar(
                                out=ms[:],
                                in0=io_t[:],
                                scalar1=dl_t[:, ci : ci + 1],
                                scalar2=None,
                                op0=mybir.AluOpType.is_equal,
                            )
                        nc.tensor.matmul(
                            A[:],
                            lhsT=ms[:],
                            rhs=gx[:, jj * in_ch : (jj + 1) * in_ch],
                            start=first,
                            stop=False,
                        )
                        first = False
                # A[d, c] += dinv[d]^2 * x[d, c] (host-prescaled), via identity mm
                xs_t = xsp.tile([P, in_ch], sc_mm_dt, tag="xs", name=f"xs_{b}")
                # ACT's HWDGE queue: splits hw-dynamic traffic off the sync queue
                nc.scalar.dma_start(
                    out=xs_t[:], in_=xs_d.ap()[b * dblk : (b + 1) * dblk, :]
                )
                nc.tensor.matmul(
                    A[:], lhsT=idr_t[:], rhs=xs_t[:], start=first, stop=True
                )
                a_s = aS.tile([P, in_ch], F32, tag="as", name=f"as_{b}")
                # PSUM->SBUF copy doubles as the dinv[dst] scale (norm factor);
                # runs on DVE (idle when Msel tiles are streamed) to keep the
                # ACT engine off the per-block critical chain
                if MSPRE:
                    nc.vector.tensor_scalar(
                        out=a_s[:],
                        in0=A[:],
                        scalar1=dsc_t[:, b : b + 1],
                        scalar2=None,
                        op0=mybir.AluOpType.mult,
                    )
                else:
                    nc.scalar.activation(
                        out=a_s[:],
                        in_=A[:],
                        func=mybir.ActivationFunctionType.Copy,
                        scale=dsc_t[:, b : b + 1],
                    )
                # transpose A halves on the PE: AT[c, d] = A[d, c]^T
                at_s = []
                for h in range(nch):
                    atp = psT.tile([P, P], F32, tag=f"atp{h}", name=f"atp{h}_{b}")
                    nc.tensor.transpose(
                        out=atp[:], in_=a_s[:, h * P : (h + 1) * P], identity=id_t[:]
                    )
                    ats = aS.tile([P, P], fin_mm_dt, tag=f"ats{h}", name=f"ats{h}_{b}")
                    if MSPRE and h == 0:
                        nc.vector.tensor_scalar(
                            out=ats[:], in0=atp[:], scalar1=1.0, scalar2=None,
                            op0=mybir.AluOpType.mult,
                        )
                    else:
                        nc.scalar.copy(ats[:], atp[:])
                    at_s.append(ats)
                Hp = hps.tile([P, hid], F32, tag="hp", name=f"hp_{b}")
                for h in range(nch):
                    nc.tensor.matmul(
                        Hp[:],
                        lhsT=fin_cast(at_s[h][:]),
                        rhs=fin_cast(wt_t[h][:]),
                        start=(h == 0),
                        stop=(not has_bias and h == nch - 1),
                    )
                if has_bias:
                    nc.tensor.matmul(
                        Hp[:],
                        lhsT=fin_cast(on_t[:]),
                        rhs=fin_cast(bs_t[:]),
                        start=False,
                        stop=True,
                    )
                os_ = outp.tile([P, hid], OUT_DT, tag="os", name=f"os_{b}")
                t2 = (
                    None
                    if PRELU_ACT
                    else outp.tile([P, hid], F32, tag="t2", name=f"t2_{b}")
                )
                if PRELU_ACT:
                    # single ACT-engine parametric relu: x>0 ? x : alpha*x
                    nc.scalar.activation(
                        out=os_[:],
                        in_=Hp[:],
                        func=mybir.ActivationFunctionType.Prelu,
                        alpha=float(alpha),
                    )
                elif 0.0 <= alpha <= 1.0:
                    # PReLU = max(H, alpha*H)
                    nc.scalar.activation(
                        out=t2[:],
                        in_=Hp[:],
                        func=mybir.ActivationFunctionType.Copy,
                        scale=float(alpha),
                    )
                    nc.vector.tensor_tensor(
                        out=os_[:], in0=t2[:], in1=Hp[:], op=mybir.AluOpType.max
                    )
                else:
                    # general PReLU: relu(H)*(1-alpha) + alpha*H
                    nc.scalar.activation(
                        out=t2[:],
                        in_=Hp[:],
                        func=mybir.ActivationFunctionType.Relu,
                    )
                    nc.vector.tensor_scalar(
                        out=t2[:],
                        in0=t2[:],
                        scalar1=float(1.0 - alpha),
                        scalar2=None,
                        op0=mybir.AluOpType.mult,
                    )
                    t3 = outp.tile([P, hid], F32, tag="t3", name=f"t3_{b}")
                    nc.vector.tensor_scalar(
                        out=t3[:],
                        in0=Hp[:],
                        scalar1=float(alpha),
                        scalar2=None,
                        op0=mybir.AluOpType.mult,
                    )
                    nc.vector.tensor_tensor(
                        out=os_[:], in0=t2[:], in1=t3[:], op=mybir.AluOpType.add
                    )
                row0 = b * dblk
                nc.sync.dma_start(
                    out=out_d.ap()[row0 : row0 + dblk, :], in_=os_[:, :]
                )
    nc.compile()
    return nc


def _make_in_maps(
    x, weight, bias, idx16, dstl, dsc, dinv, gperm,
    sc_np=np.float32, ncores=NCORES,
):
    dstl, msall = dstl
    x = np.asarray(x, dtype=np.float32)
    w = np.asarray(weight, dtype=np.float32)
    n = x.shape[0]
    half = n // 2
    in_ch = x.shape[1]
    hid = w.shape[0]
    npc = n // ncores
    bpc = (npc + P - 1) // P
    npc_pad = bpc * P
    iota = np.tile(np.arange(P, dtype=sc_np), (P, 1))
    wts = {
        f"wt{h}": np.ascontiguousarray(w[:, h * P : (h + 1) * P].T)
        for h in range(in_ch // P)
    }
    bias_row = np.asarray(bias, dtype=np.float32).reshape(1, hid)
    xp = x * dinv[:, None]  # dinv[src] premultiplied into the gathered rows
    xlo = np.ascontiguousarray(xp[:half].astype(sc_np))
    xhi = np.ascontiguousarray(xp[half:].astype(sc_np))
    xself_all = xp  # self term: dinv*x here, the copy scale adds dinv[dst]
    in_maps = []
    for k in range(ncores):
        xs = np.zeros((npc_pad, in_ch), sc_np)
        loc = gperm[k * npc : (k + 1) * npc] - k * npc_pad
        xs[loc] = xself_all[k * npc : (k + 1) * npc].astype(sc_np)
        m = {
            "x0": xlo,
            "x1": xhi,
            "idx16": np.ascontiguousarray(idx16[k]),
            "dsc": np.ascontiguousarray(dsc[k]),
            "iota": iota,
            "xself": xs,
            "bias": bias_row,
            "ones": np.ones((1, P), np.float32),
            "idr": np.eye(P, dtype=sc_np),
        }
        if msall is not None:
            m["msall"] = np.ascontiguousarray(msall[k])
        else:
            m["dstl"] = np.ascontiguousarray(dstl[k])
            m["dlneg"] = np.ascontiguousarray(-dstl[k])
        m.update(wts)
        in_maps.append(m)
    return in_maps


# Results of the last kernel() call, for the test harness.
LAST_RESULTS = None


def _dt_opts():
    sc = os.environ.get("GCN_SC_DT", "bf16")
    fin = os.environ.get("GCN_FIN_DT", "f32r")
    sc_dt = {"f32": F32, "f32r": F32, "bf16": BF16}[sc]
    sc_mm_dt = {"f32": F32, "f32r": mybir.dt.float32r, "bf16": BF16}[sc]
    fin_mm_dt = {"f32": F32, "f32r": mybir.dt.float32r}[fin]
    sc_np = np.float32 if sc_dt == F32 else mybir.dt.np(BF16)
    return sc_dt, sc_mm_dt, fin_mm_dt, sc_np


def kernel(x, edge_index, weight, bias, prelu_a):
    global LAST_RESULTS
    sc_dt, sc_mm_dt, fin_mm_dt, sc_np = _dt_opts()
    trace = os.environ.get("GCN_TRACE", "0") == "1"

    klo, khi, idx16, dstl, dsc, dinv, gperm = _preprocess(edge_index)
    alpha = float(np.asarray(prelu_a).ravel()[0])
    has_bias = bool(np.any(np.asarray(bias)))
    nc = _build_program(
        klo, khi, alpha, sc_dt=sc_dt, sc_mm_dt=sc_mm_dt, fin_mm_dt=fin_mm_dt,
        has_bias=has_bias,
    )
    in_maps = _make_in_maps(
        x, weight, bias, idx16, dstl, dsc, dinv, gperm, sc_np=sc_np
    )

    res = bass_utils.run_bass_kernel_spmd(
        nc, in_maps, core_ids=list(range(NCORES)), trace=trace
    )
    LAST_RESULTS = res
    out = np.concatenate([res.results[k]["out"] for k in range(NCORES)], axis=0)
    return out[gperm].astype(np.float32)


# revision 17
# speedup vs baseline: 1.2904x; 1.0104x over previous
"""GCN encoder (GCNConv + PReLU) as a Bass/Tile kernel on 8 Trainium2 NeuronCores.

Math (matches PyG GCNConv with self-loops + symmetric norm, then PReLU):
    deg[i]  = in-degree of i over dst (+1 self loop)
    dinv    = 1/sqrt(deg)
    agg[d]  = sum_{e:(s->d)} dinv[s]*dinv[d] * x[s] + dinv[d]^2 * x[d]
    out     = PReLU(agg @ W.T + bias)

Distribution: dst-node sharding, core k owns nodes [k*6250, (k+1)*6250).

Per-core pipeline (dst-blocks of 128 nodes):
  - each core's 6250 dst nodes are packed into 49 blocks of <=128 by a 2D
    bin-packing pass on (lo-half, hi-half) in-degree, so nearly every
    (block, src-half) cell fits exactly 4 chunks of 128 edges (~398 chunks
    vs 490 under the natural contiguous assignment). Chunk capacities are
    shared across cores (one SPMD program); a few data-driven spill blocks
    get 5 chunks. Block-permuted output rows are un-permuted on the host.
  - the symmetric norm factorizes: dinv[src] is premultiplied into the
    shipped x halves on the host; dinv[dst] is applied on-chip as the
    per-partition scale of the PSUM->SBUF copy of each block's accumulator.
    The self-loop term ships as dinv*x rows and rides the same copy scale.
  - non-self edges are grouped by (dst-block, src-half) on the host and
    packed into 128-edge chunks; src rows are fetched in bf16 with
    `dma_gather` (int16 indices => x is split into two 25000-row halves).
    Gathers rotate over 4 SWDGE queues, GBLK=3 blocks per gather pair.
    Pad slots gather row 0; their Msel column is zero so they contribute 0.
  - the scatter-add runs on the PE: per chunk, a one-hot selection matrix
    Msel[e, d] = (d == dst_local[e]) turns it into one matmul per chunk,
    A[d, c] += Msel[e, d]^T @ gx[e, c], accumulated in PSUM. Msel tiles are
    PRECOMPUTED on the host (exact one-hot bf16) and streamed per group on
    the sync HWDGE queue -- building them on DVE cost ~550ns/chunk of
    vector-engine time (~220us/core), which made DVE a co-bottleneck with
    the gather descriptor generation on GpSimd. Streaming costs ~13MB/core
    of extra DMA but leaves the Pool engine as the only pacer.
  - A is transposed with the PE (128x128 via identity); the PSUM->SBUF
    copies are split between DVE and ACT so neither sits on the critical
    per-block chain. The weight matmul H = A^T W^T (+ ones^T bias when the
    bias is nonzero -- added after the dinv[dst] scale, so correct) then a
    single ACT parametric-relu writes the bf16 output tile.
  - the gather-descriptor generation on the Pool engine (SWDGE ucode,
    ~2-4.5ns/row) is the end-to-end pacer; everything else overlaps it.

Dtype knobs (env):
  GCN_SC_DT  = f32 | f32r | bf16   scatter path (gather + Msel + edge matmul)
  GCN_FIN_DT = f32 | f32r          weight matmul path
  GCN_OUT_DT = f32 | bf16          DRAM output tile
Defaults (bf16 gather/Msel/out, f32r weight matmul, GBLK=3 tapered groups,
streamed Msel) measure ~193us on hardware vs ~435us baseline;
rel err ~2.3e-3 (bf16 quantization of x), tolerance 2e-2.
"""

import os
import numpy as np
from contextlib import ExitStack

import concourse.bass as bass
import concourse.tile as tile
from concourse import bacc, mybir, bass_utils
from concourse.masks import make_identity

# Problem shape (fixed by the harness contract).
N_NODES = 50000
N_EDGES = 400000
IN_CH = 256
HID = 512
NCORES = 8
NPC = N_NODES // NCORES  # dst nodes owned per core
P = 128
BPC = (NPC + P - 1) // P  # dst blocks per core
NPC_PAD = BPC * P

F32 = mybir.dt.float32
BF16 = mybir.dt.bfloat16
# blocks whose gathers are merged into one dma_gather pair (lo/hi)
GBLK = int(os.environ.get("GCN_GBLK", "3"))
# of every 8 Msel builds, this many go to the scalar engine (rest on vector)
MSACT = int(os.environ.get("GCN_MSACT", "0"))
# pack each dma_gather's descriptors into one packet (crashes NRT for big
# gathers — exceeds the 64-descriptor packet ceiling; keep 0)
SINGLE_PACKET = os.environ.get("GCN_SINGLE_PACKET", "0") == "1"
# number of SWDGE queues to rotate gathers over (ucode max 4)
NQUEUES = int(os.environ.get("GCN_NQ", "4"))
# use the ACT engine's parametric relu (1 op) for PReLU
PRELU_ACT = os.environ.get("GCN_PRELU_ACT", "1") == "1"
# dtype of the DRAM output tile (bf16 halves store traffic; host re-widens)
OUT_DT = {"f32": F32, "bf16": BF16}[os.environ.get("GCN_OUT_DT", "bf16")]
# stream host-precomputed one-hot Msel tiles instead of building them on DVE
MSPRE = os.environ.get("GCN_MSPRE", "1") == "1"


def _group_plan(bpc):
    """Blocks per gather group: small groups at both ends (fast pipeline
    fill, little leftover work after the last gather), GBLK-sized in the
    middle. Shared by _preprocess and _build_program."""
    if os.environ.get("GCN_TAPER", "1") != "1" or GBLK <= 1:
        return [min(GBLK, bpc - i) for i in range(0, bpc, GBLK)]
    head = [1, 1, 2]
    tail = [2, 1, 1]
    mid_n = bpc - sum(head) - sum(tail)
    if mid_n <= 0:
        return [min(GBLK, bpc - i) for i in range(0, bpc, GBLK)]
    mid = [GBLK] * (mid_n // GBLK)
    if mid_n % GBLK:
        mid.append(mid_n % GBLK)
    return head + mid + tail


def _gx_bufs(sc_mm_dt):
    env = os.environ.get("GCN_GXBUFS")
    if env:
        return int(env)
    return 4 if sc_mm_dt == BF16 else max(2, 8 // GBLK)


# SWDGE descriptor-ring carveout in SBUF (bytes)
DMA_SCRATCH = int(os.environ.get("GCN_DMA_SCRATCH", "32768"))


def _pack_core(dlo, dhi, caplo, caphi, sizes):
    """Assign nodes (with per-half degree pairs) to blocks with per-block
    degree capacities and seat counts. Snake init + greedy swap repair.
    Returns blk[node] or None if the repair got stuck."""
    npc = len(dlo)
    nb = len(caplo)
    order = np.argsort(-(dlo + dhi), kind="stable")
    blk = np.empty(npc, np.int32)
    seat_left = sizes.copy()
    bi, direction = 0, 1
    for n in order:
        for _ in range(2 * nb):
            if seat_left[bi] > 0:
                break
            bi += direction
            if bi == nb:
                bi, direction = nb - 1, -1
            elif bi < 0:
                bi, direction = 0, 1
        blk[n] = bi
        seat_left[bi] -= 1
        bi += direction
        if bi == nb:
            bi, direction = nb - 1, -1
        elif bi < 0:
            bi, direction = 0, 1
    for _ in range(6000):
        slo = np.bincount(blk, weights=dlo, minlength=nb)
        shi = np.bincount(blk, weights=dhi, minlength=nb)
        vlo = slo - caplo
        vhi = shi - caphi
        viol = np.maximum(vlo, 0) + np.maximum(vhi, 0)
        if viol.max() <= 0:
            return blk
        B = int(np.argmax(viol))
        in_b = np.where(blk == B)[0]
        w_u = dlo[in_b] * (vlo[B] > 0) + dhi[in_b] * (vhi[B] > 0)
        u = in_b[np.argmax(w_u)]
        cand = np.where(blk != B)[0]
        C = blk[cand]
        dl_u, dh_u = dlo[u], dhi[u]
        nlo_b = slo[B] - dl_u + dlo[cand]
        nhi_b = shi[B] - dh_u + dhi[cand]
        nlo_c = slo[C] + dl_u - dlo[cand]
        nhi_c = shi[C] + dh_u - dhi[cand]
        newv = (
            np.maximum(nlo_b - caplo[B], 0)
            + np.maximum(nhi_b - caphi[B], 0)
            + np.maximum(nlo_c - caplo[C], 0)
            + np.maximum(nhi_c - caphi[C], 0)
        )
        gain = (viol[B] + viol[C]) - newv
        j = int(np.argmax(gain))
        if gain[j] <= 0:
            return None
        v = cand[j]
        blk[u], blk[v] = blk[v], B
    return None


def _preprocess(edge_index, n_nodes=N_NODES, ncores=NCORES):
    """Balance dst nodes into blocks, then group non-self edges by
    (core, dst-block, src-half) and pack into 128-edge chunks (capacities
    shared across cores so all cores run one program).

    Returns (klo, khi, idx16, dstl, nrm, dinv, gperm):
      klo/khi: per-block chunk counts for the lo/hi gathers (compile-time)
      idx16:   [ncores, 128, 8*tot] int16 gather indices (16-wrap, 8x tiled)
      dstl:    [ncores, 128, tot] f32 dst position-in-block per edge slot
      nrm:     [ncores, 128, tot] f32 edge norm (0 on padded slots)
      dinv:    [n_nodes] f32 1/sqrt(deg)
      gperm:   [n_nodes] int64; node v's output row is concat-out[gperm[v]]
    """
    npc = n_nodes // ncores
    half = n_nodes // 2
    bpc = (npc + P - 1) // P
    src = np.asarray(edge_index[0]).astype(np.int64).ravel()
    dst = np.asarray(edge_index[1]).astype(np.int64).ravel()
    deg = np.bincount(dst, minlength=n_nodes).astype(np.float32) + 1.0
    dinv = (1.0 / np.sqrt(deg)).astype(np.float32)
    n_all = dinv[src] * dinv[dst]

    core = dst // npc
    dloc = dst - core * npc
    hi = (src >= half).astype(np.int64)

    # per-core degree pairs over (lo, hi) src halves
    dlo_all = np.zeros((ncores, npc), np.int64)
    dhi_all = np.zeros((ncores, npc), np.int64)
    np.add.at(dlo_all, (core[hi == 0], dloc[hi == 0]), 1)
    np.add.at(dhi_all, (core[hi == 1], dloc[hi == 1]), 1)

    sizes = np.full(bpc, P, np.int64)
    extra = bpc * P - npc
    if extra:
        sizes[-extra:] = P - 1  # tail blocks hold one fewer node

    tmax = max(int(dlo_all.sum(1).max()), int(dhi_all.sum(1).max()))
    spill = max(0, -(-(tmax - bpc * 4 * P) // P)) + 1
    while True:
        caps = np.full(bpc, 4, np.int64)
        for i in range(spill):
            caps[i % bpc] += 1
        caplo = caps * P
        caphi = caps * P
        blks = []
        for k in range(ncores):
            b = _pack_core(dlo_all[k], dhi_all[k], caplo, caphi, sizes)
            if b is None:
                break
            blks.append(b)
        if len(blks) == ncores:
            break
        spill += 2
    klo = [int(c) for c in caps]
    khi = [int(c) for c in caps]

    # node -> (block, position) and the global output permutation
    blk_of = np.empty(n_nodes, np.int64)
    pos_of = np.empty(n_nodes, np.int64)
    for k in range(ncores):
        b = blks[k].astype(np.int64)
        order = np.lexsort((np.arange(npc), b))
        pos = np.empty(npc, np.int64)
        bsorted = b[order]
        start = np.searchsorted(bsorted, np.arange(bpc))
        pos[order] = np.arange(npc) - start[bsorted]
        blk_of[k * npc : (k + 1) * npc] = b
        pos_of[k * npc : (k + 1) * npc] = pos
    gperm = (dst_core := np.arange(n_nodes) // npc) * (bpc * P) + blk_of * P + pos_of
    del dst_core

    kblk = [a + b for a, b in zip(klo, khi)]
    tot = sum(kblk)

    blk = blk_of[dst]
    key = (core * bpc + blk) * 2 + hi
    nkeys = ncores * bpc * 2
    counts = np.bincount(key, minlength=nkeys).reshape(ncores, bpc, 2)
    assert (counts[:, :, 0] <= caplo[None, :]).all()
    assert (counts[:, :, 1] <= caphi[None, :]).all()

    order = np.argsort(key, kind="stable")
    key_sorted = key[order]
    grp_start = np.zeros(nkeys + 1, np.int64)
    grp_start[1:] = np.cumsum(counts.reshape(-1))
    rank = np.arange(len(key_sorted)) - grp_start[key_sorted]

    # chunk layout groups GBLK consecutive blocks per gather pair:
    # [lo(b0) lo(b1) .. | hi(b0) hi(b1) ..] per group, groups consecutive
    segbase = np.zeros((bpc, 2), np.int64)
    off = 0
    g0 = 0
    for gsz in _group_plan(bpc):
        blocks = range(g0, g0 + gsz)
        g0 += gsz
        for b in blocks:
            segbase[b, 0] = off
            off += klo[b]
        for b in blocks:
            segbase[b, 1] = off
            off += khi[b]
    assert off == tot

    ob, oh, oc = blk[order], hi[order], core[order]
    base = segbase[ob, oh]
    ck = base + rank // P
    pp = rank % P

    # pad slots keep dstl=200 (matches no iota value -> zero Msel column);
    # edge norms factorize: dinv[src] is folded into the shipped x, dinv[dst]
    # into the per-block PSUM->SBUF copy scale (dsc).
    dstl = np.full((ncores, P, tot), 200.0, np.float32)
    dstl[oc, pp, ck] = pos_of[dst[order]].astype(np.float32)
    dsc = np.zeros((ncores, P, bpc), np.float32)
    nodes = np.arange(n_nodes)
    dsc[nodes // npc, pos_of, blk_of] = dinv
    if MSPRE:
        # dense one-hot Msel, streamed per group at runtime
        import ml_dtypes
        msall = np.zeros((ncores, P, tot * P), ml_dtypes.bfloat16)
        msall[oc, pp, ck * P + pos_of[dst[order]]] = 1.0
    else:
        msall = None

    del n_all
    s16 = (src[order] - oh * half).astype(np.int16)
    col = 8 * base + (rank // 16)
    row = rank % 16
    idx16 = np.zeros((ncores, 16, 8 * tot), np.int16)  # pads gather row 0
    idx16[oc, row, col] = s16
    idx16 = np.tile(idx16, (1, 8, 1))
    return klo, khi, idx16, (dstl, msall), dsc, dinv, gperm


def _build_program(
    klo,
    khi,
    alpha,
    sc_dt=F32,
    sc_mm_dt=None,
    fin_mm_dt=None,
    n_nodes=N_NODES,
    ncores=NCORES,
    in_ch=IN_CH,
    hid=HID,
    has_bias=True,
):
    """Build the per-core Bass program (identical across cores).

    sc_dt: storage dtype of gather/Msel tiles (F32 or BF16).
    sc_mm_dt: dtype the scatter matmul sees (defaults to sc_dt; use
        mybir.dt.float32r with sc_dt=F32 for fast near-fp32 matmuls).
    fin_mm_dt: dtype of the weight matmul (F32 or float32r).
    """
    dblk = P
    npc = n_nodes // ncores
    half = n_nodes // 2
    bpc = len(klo)
    kblk = [a + b for a, b in zip(klo, khi)]
    tot = sum(kblk)
    nch = in_ch // P
    npc_pad = bpc * dblk
    sc_mm_dt = sc_mm_dt or sc_dt
    fin_mm_dt = fin_mm_dt or F32

    def fin_cast(ap):
        return ap

    nc = bacc.Bacc(
        "TRN2", target_bir_lowering=False, debug=False,
        num_swdge_queues=NQUEUES, dynamic_dma_scratch_size=DMA_SCRATCH,
    )

    x_ds = [
        nc.dram_tensor(f"x{h}", [half, in_ch], sc_mm_dt, kind="ExternalInput")
        for h in range(2)
    ]
    si_d = nc.dram_tensor("idx16", [P, 8 * tot], mybir.dt.int16, kind="ExternalInput")
    if MSPRE:
        ms_d = nc.dram_tensor("msall", [P, tot * P], sc_dt, kind="ExternalInput")
    else:
        dl_d = nc.dram_tensor("dstl", [P, tot], F32, kind="ExternalInput")
        dln_d = nc.dram_tensor("dlneg", [P, tot], F32, kind="ExternalInput")
    dsc_d = nc.dram_tensor("dsc", [P, bpc], F32, kind="ExternalInput")
    io_d = nc.dram_tensor("iota", [P, dblk], sc_mm_dt, kind="ExternalInput")
    xs_d = nc.dram_tensor("xself", [npc_pad, in_ch], sc_mm_dt, kind="ExternalInput")
    wt_ds = [
        nc.dram_tensor(f"wt{h}", [P, hid], fin_mm_dt, kind="ExternalInput")
        for h in range(nch)
    ]
    bs_d = nc.dram_tensor("bias", [1, hid], fin_mm_dt, kind="ExternalInput")
    on_d = nc.dram_tensor("ones", [1, P], fin_mm_dt, kind="ExternalInput")
    idr_d = nc.dram_tensor("idr", [P, P], sc_mm_dt, kind="ExternalInput")
    out_d = nc.dram_tensor("out", [npc_pad, hid], OUT_DT, kind="ExternalOutput")

    with tile.TileContext(nc) as tc, ExitStack() as ctx:
        const = ctx.enter_context(tc.tile_pool(name="const", bufs=1))
        gx_bufs = _gx_bufs(sc_mm_dt)
        gxp = ctx.enter_context(tc.tile_pool(name="gx", bufs=gx_bufs))
        mselp = ctx.enter_context(tc.tile_pool(name="msel", bufs=8))
        psA = ctx.enter_context(tc.tile_pool(name="psA", bufs=3, space="PSUM"))
        psT = ctx.enter_context(tc.tile_pool(name="psT", bufs=1, space="PSUM"))
        hps = ctx.enter_context(tc.tile_pool(name="hps", bufs=3, space="PSUM"))
        aS = ctx.enter_context(tc.tile_pool(name="aS", bufs=4))
        xsp = ctx.enter_context(tc.tile_pool(name="xsp", bufs=4))
        outp = ctx.enter_context(tc.tile_pool(name="outp", bufs=6))

        # column-chunked const loads: subtile deps let the first gathers and
        # Msel builds start as soon as their slice lands, not the whole 1MB
        def _chunked_load(t, dsrc, ncols, npiece=8, taper=False):
            bounds = [0]
            if taper:
                # first pieces tiny: the first gathers only need a sliver
                bounds += [max(1, ncols // 64), max(2, ncols // 16)]
            step = -(-(ncols - bounds[-1]) // npiece)
            a = bounds[-1]
            while a < ncols:
                a = min(a + step, ncols)
                bounds.append(a)
            for a, b in zip(bounds, bounds[1:]):
                if b > a:
                    nc.sync.dma_start(out=t[:, a:b], in_=dsrc.ap()[:, a:b])

        si_t = const.tile([P, 8 * tot], mybir.dt.int16)
        _chunked_load(si_t, si_d, 8 * tot,
                      taper=os.environ.get("GCN_SITAPER", "0") == "1")
        if not MSPRE:
            dl_t = const.tile([P, tot], F32)
            _chunked_load(dl_t, dl_d, tot)
            if MSACT > 0:
                dln_t = const.tile([P, tot], F32)
                _chunked_load(dln_t, dln_d, tot)
        dsc_t = const.tile([P, bpc], F32)
        nc.sync.dma_start(out=dsc_t[:], in_=dsc_d.ap())
        io_t = const.tile([P, dblk], sc_mm_dt)
        nc.sync.dma_start(out=io_t[:], in_=io_d.ap())
        wt_t = []
        for h in range(nch):
            w = const.tile([P, hid], fin_mm_dt, name=f"wt_t{h}")
            nc.sync.dma_start(out=w[:], in_=wt_ds[h].ap())
            wt_t.append(w)
        bs_t = const.tile([1, hid], fin_mm_dt)
        nc.sync.dma_start(out=bs_t[:], in_=bs_d.ap())
        on_t = const.tile([1, P], fin_mm_dt)
        nc.sync.dma_start(out=on_t[:], in_=on_d.ap())
        id_t = const.tile([P, P], F32)
        make_identity(nc, id_t[:])
        idr_t = const.tile([P, P], sc_mm_dt)
        nc.sync.dma_start(out=idr_t[:], in_=idr_d.ap())

        # group-level chunk bases (same layout as _preprocess)
        segbase = np.zeros((bpc, 2), np.int64)
        off = 0
        groups = []
        g0 = 0
        for gsz in _group_plan(bpc):
            blocks = list(range(g0, g0 + gsz))
            g0 += gsz
            for b in blocks:
                segbase[b, 0] = off
                off += klo[b]
            for b in blocks:
                segbase[b, 1] = off
                off += khi[b]
            groups.append(blocks)

        # fixed slab extent so the pool rotates uniform buffers
        gmax = [
            max(sum((klo, khi)[h][b] for b in blocks) for blocks in groups)
            for h in range(2)
        ]

        gather_qn = 0
        for gri, blocks in enumerate(groups):
            b0 = blocks[0]
            kg = [sum(klo[b] for b in blocks), sum(khi[b] for b in blocks)]
            gstart = [int(segbase[b0, 0]), int(segbase[b0, 1])]
            gxs = []
            for h in range(2):
                if kg[h] == 0:
                    gxs.append(None)
                    continue
                nidx = kg[h] * P
                gx = gxp.tile(
                    [P, gmax[h] * in_ch], sc_mm_dt, tag=f"gx{h}", name=f"gx{h}_{b0}"
                )
                nc.gpsimd.dma_gather(
                    gx[:, : kg[h] * in_ch].rearrange("p (k d) -> p k d", d=in_ch),
                    x_ds[h].ap(),
                    si_t[:, 8 * gstart[h] : 8 * (gstart[h] + kg[h])],
                    nidx,
                    nidx,
                    in_ch,
                    queue_num=gather_qn % NQUEUES,
                    single_packet=SINGLE_PACKET,
                )
                gather_qn += 1
                gxs.append(gx)
            if MSPRE:
                # one streamed load covers the group's lo+hi chunk run
                g_c0 = gstart[0]
                g_nc = kg[0] + kg[1]
                msg = mselp.tile(
                    [P, g_nc * P], sc_dt, tag="msg", name=f"msg_{b0}"
                )
                msq = nc.scalar if os.environ.get("GCN_MSGQ") == "act" else nc.sync
                msq.dma_start(
                    out=msg[:], in_=ms_d.ap()[:, g_c0 * P : (g_c0 + g_nc) * P]
                )
            for b in blocks:
                A = psA.tile([P, in_ch], F32, tag="A", name=f"A_{b}")
                first = True
                for h in range(2):
                    gx = gxs[h]
                    koff = int(segbase[b, h]) - gstart[h]
                    kk = (klo, khi)[h][b]
                    for j in range(kk):
                        ci = int(segbase[b, h]) + j
                        jj = koff + j
                        if MSPRE:
                            ms = msg[:, (ci - g_c0) * P : (ci - g_c0 + 1) * P]
                            nc.tensor.matmul(
                                A[:],
                                lhsT=ms,
                                rhs=gx[:, jj * in_ch : (jj + 1) * in_ch],
                                start=first,
                                stop=False,
                            )
                            first = False
                            continue
                        ms = mselp.tile(
                            [P, dblk], sc_mm_dt, tag="ms", name=f"ms_{b}_{h}_{j}"
                        )
                        if ci % 8 < MSACT:
                            # ScalarE build: ms = relu(1 - |iota - dst|)
                            mt = mselp.tile(
                                [P, dblk], sc_mm_dt, tag="mt", name=f"mt_{b}_{h}_{j}"
                            )
                            nc.scalar.activation(
                                out=mt[:],
                                in_=io_t[:],
                                func=mybir.ActivationFunctionType.Abs,
                                bias=dln_t[:, ci : ci + 1],
                            )
                            nc.scalar.activation(
                                out=ms[:],
                                in_=mt[:],
                                func=mybir.ActivationFunctionType.Relu,
                                scale=-1.0,
                                bias=1.0,
                            )
                        else:
                            nc.vector.tensor_scalar(
                                out=ms[:],
                                in0=io_t[:],
                                scalar1=dl_t[:, ci : ci + 1],
                                scalar2=None,
                                op0=mybir.AluOpType.is_equal,
                            )
                        nc.tensor.matmul(
                            A[:],
                            lhsT=ms[:],
                            rhs=gx[:, jj * in_ch : (jj + 1) * in_ch],
                            start=first,
                            stop=False,
                        )
                        first = False
                # A[d, c] += dinv[d]^2 * x[d, c] (host-prescaled), via identity mm
                xs_t = xsp.tile([P, in_ch], sc_mm_dt, tag="xs", name=f"xs_{b}")
                # ACT's HWDGE queue: splits hw-dynamic traffic off the sync queue
                nc.scalar.dma_start(
                    out=xs_t[:], in_=xs_d.ap()[b * dblk : (b + 1) * dblk, :]
                )
                nc.tensor.matmul(
                    A[:], lhsT=idr_t[:], rhs=xs_t[:], start=first, stop=True
                )
                a_s = aS.tile([P, in_ch], F32, tag="as", name=f"as_{b}")
                # PSUM->SBUF copy doubles as the dinv[dst] scale (norm factor);
                # runs on DVE (idle when Msel tiles are streamed) to keep the
                # ACT engine off the per-block critical chain
                if MSPRE:
                    nc.vector.tensor_scalar(
                        out=a_s[:],
                        in0=A[:],
                        scalar1=dsc_t[:, b : b + 1],
                        scalar2=None,
                        op0=mybir.AluOpType.mult,
                    )
                else:
                    nc.scalar.activation(
                        out=a_s[:],
                        in_=A[:],
                        func=mybir.ActivationFunctionType.Copy,
                        scale=dsc_t[:, b : b + 1],
                    )
                # transpose A halves on the PE: AT[c, d] = A[d, c]^T
                at_s = []
                for h in range(nch):
                    atp = psT.tile([P, P], F32, tag=f"atp{h}", name=f"atp{h}_{b}")
                    nc.tensor.transpose(
                        out=atp[:], in_=a_s[:, h * P : (h + 1) * P], identity=id_t[:]
                    )
                    ats = aS.tile([P, P], fin_mm_dt, tag=f"ats{h}", name=f"ats{h}_{b}")
                    if MSPRE and h == 0:
                        nc.vector.tensor_scalar(
                            out=ats[:], in0=atp[:], scalar1=1.0, scalar2=None,
                            op0=mybir.AluOpType.mult,
                        )
                    else:
                        nc.scalar.copy(ats[:], atp[:])
                    at_s.append(ats)
                Hp = hps.tile([P, hid], F32, tag="hp", name=f"hp_{b}")
                for h in range(nch):
                    nc.tensor.matmul(
                        Hp[:],
                        lhsT=fin_cast(at_s[h][:]),
                        rhs=fin_cast(wt_t[h][:]),
                        start=(h == 0),
                        stop=(not has_bias and h == nch - 1),
                    )
                if has_bias:
                    nc.tensor.matmul(
                        Hp[:],
                        lhsT=fin_cast(on_t[:]),
                        rhs=fin_cast(bs_t[:]),
                        start=False,
                        stop=True,
                    )
                os_ = outp.tile([P, hid], OUT_DT, tag="os", name=f"os_{b}")
                t2 = (
                    None
                    if PRELU_ACT
                    else outp.tile([P, hid], F32, tag="t2", name=f"t2_{b}")
                )
                if PRELU_ACT:
                    # single ACT-engine parametric relu: x>0 ? x : alpha*x
                    nc.scalar.activation(
                        out=os_[:],
                        in_=Hp[:],
                        func=mybir.ActivationFunctionType.Prelu,
                        alpha=float(alpha),
                    )
                elif 0.0 <= alpha <= 1.0:
                    # PReLU = max(H, alpha*H)
                    nc.scalar.activation(
                        out=t2[:],
                        in_=Hp[:],
                        func=mybir.ActivationFunctionType.Copy,
                        scale=float(alpha),
                    )
                    nc.vector.tensor_tensor(
                        out=os_[:], in0=t2[:], in1=Hp[:], op=mybir.AluOpType.max
                    )
                else:
                    # general PReLU: relu(H)*(1-alpha) + alpha*H
                    nc.scalar.activation(
                        out=t2[:],
                        in_=Hp[:],
                        func=mybir.ActivationFunctionType.Relu,
                    )
                    nc.vector.tensor_scalar(
                        out=t2[:],
                        in0=t2[:],
                        scalar1=float(1.0 - alpha),
                        scalar2=None,
                        op0=mybir.AluOpType.mult,
                    )
                    t3 = outp.tile([P, hid], F32, tag="t3", name=f"t3_{b}")
                    nc.vector.tensor_scalar(
                        out=t3[:],
                        in0=Hp[:],
                        scalar1=float(alpha),
                        scalar2=None,
                        op0=mybir.AluOpType.mult,
                    )
                    nc.vector.tensor_tensor(
                        out=os_[:], in0=t2[:], in1=t3[:], op=mybir.AluOpType.add
                    )
                row0 = b * dblk
                nc.sync.dma_start(
                    out=out_d.ap()[row0 : row0 + dblk, :], in_=os_[:, :]
                )
    nc.compile()
    return nc


def _make_in_maps(
    x, weight, bias, idx16, dstl, dsc, dinv, gperm,
    sc_np=np.float32, ncores=NCORES,
):
    dstl, msall = dstl
    x = np.asarray(x, dtype=np.float32)
    w = np.asarray(weight, dtype=np.float32)
    n = x.shape[0]
    half = n // 2
    in_ch = x.shape[1]
    hid = w.shape[0]
    npc = n // ncores
    bpc = (npc + P - 1) // P
    npc_pad = bpc * P
    iota = np.tile(np.arange(P, dtype=sc_np), (P, 1))
    wts = {
        f"wt{h}": np.ascontiguousarray(w[:, h * P : (h + 1) * P].T)
        for h in range(in_ch // P)
    }
    bias_row = np.asarray(bias, dtype=np.float32).reshape(1, hid)
    xp = x * dinv[:, None]  # dinv[src] premultiplied into the gathered rows
    xlo = np.ascontiguousarray(xp[:half].astype(sc_np))
    xhi = np.ascontiguousarray(xp[half:].astype(sc_np))
    xself_all = xp  # self term: dinv*x here, the copy scale adds dinv[dst]
    in_maps = []
    for k in range(ncores):
        xs = np.zeros((npc_pad, in_ch), sc_np)
        loc = gperm[k * npc : (k + 1) * npc] - k * npc_pad
        xs[loc] = xself_all[k * npc : (k + 1) * npc].astype(sc_np)
        m = {
            "x0": xlo,
            "x1": xhi,
            "idx16": np.ascontiguousarray(idx16[k]),
            "dsc": np.ascontiguousarray(dsc[k]),
            "iota": iota,
            "xself": xs,
            "bias": bias_row,
            "ones": np.ones((1, P), np.float32),
            "idr": np.eye(P, dtype=sc_np),
        }
        if msall is not None:
            m["msall"] = np.ascontiguousarray(msall[k])
        else:
            m["dstl"] = np.ascontiguousarray(dstl[k])
            m["dlneg"] = np.ascontiguousarray(-dstl[k])
        m.update(wts)
        in_maps.append(m)
    return in_maps


# Results of the last kernel() call, for the test harness.
LAST_RESULTS = None


def _dt_opts():
    sc = os.environ.get("GCN_SC_DT", "bf16")
    fin = os.environ.get("GCN_FIN_DT", "f32r")
    sc_dt = {"f32": F32, "f32r": F32, "bf16": BF16}[sc]
    sc_mm_dt = {"f32": F32, "f32r": mybir.dt.float32r, "bf16": BF16}[sc]
    fin_mm_dt = {"f32": F32, "f32r": mybir.dt.float32r}[fin]
    sc_np = np.float32 if sc_dt == F32 else mybir.dt.np(BF16)
    return sc_dt, sc_mm_dt, fin_mm_dt, sc_np


def kernel(x, edge_index, weight, bias, prelu_a):
    global LAST_RESULTS
    sc_dt, sc_mm_dt, fin_mm_dt, sc_np = _dt_opts()
    trace = os.environ.get("GCN_TRACE", "0") == "1"

    klo, khi, idx16, dstl, dsc, dinv, gperm = _preprocess(edge_index)
    alpha = float(np.asarray(prelu_a).ravel()[0])
    has_bias = bool(np.any(np.asarray(bias)))
    nc = _build_program(
        klo, khi, alpha, sc_dt=sc_dt, sc_mm_dt=sc_mm_dt, fin_mm_dt=fin_mm_dt,
        has_bias=has_bias,
    )
    in_maps = _make_in_maps(
        x, weight, bias, idx16, dstl, dsc, dinv, gperm, sc_np=sc_np
    )

    res = bass_utils.run_bass_kernel_spmd(
        nc, in_maps, core_ids=list(range(NCORES)), trace=trace
    )
    LAST_RESULTS = res
    out = np.concatenate([res.results[k]["out"] for k in range(NCORES)], axis=0)
    return out[gperm].astype(np.float32)
